# revision 38
# baseline (speedup 1.0000x reference)
"""Quantized-AlexNet forward on 8 trn2 NeuronCores.

Strategy (v2, bf16):
  - data-parallel conv stack: 8 images per core
  - the whole matmul path runs in bf16: quantized weights are exactly
    representable (+-1 signs), activation rounding ~0.1%/layer, well
    inside the 2e-2 budget. bf16 gives 4x on conv1 (vs fp32 matmul),
    2x LDWEIGHTS via FWL, and halves all DMA traffic.
  - conv1: host im2col (bf16), 2 DMA chunks/image
  - conv2..5: shifted-window matmuls, DoReFa scale E + BN folded into
    fp32 epilogue scale/bias (ACT engine), outputs written bf16
  - maxpools: 2-pass DVE pool
  - FC: tensor-parallel over outputs; E1/E2 folded algebraically so
    epilogues stay cheap; collectives batched (2 AG for h halves
    overlapped with conv5, 1 AG each for y1/y2); fw1 prefetched on the
    gpsimd queue during the conv stack.
"""

import os
import numpy as np
import ml_dtypes

BF = ml_dtypes.bfloat16
NCORES = 8
B = 64
BPC = B // NCORES  # images per core

_PROG_CACHE = {}
LAST_EXEC_NS = None
LAST_RESULTS = None


def _build_program():
    import concourse.bass as bass
    import concourse.mybir as mybir
    import concourse.tile as tile
    from concourse import bacc
    from concourse.masks import make_identity

    F32 = mybir.dt.float32
    BF16 = mybir.dt.bfloat16
    AF = mybir.ActivationFunctionType

    def mkap(tile_ap, offset_elems, dims):
        part = tile_ap.ap[0]
        return bass.AP(
            tensor=tile_ap.tensor,
            offset=tile_ap.offset + offset_elems,
            ap=[list(part)] + [list(d) for d in dims],
        )

    def rawap(tile_ap, offset_elems, dims):
        return bass.AP(
            tensor=tile_ap.tensor,
            offset=tile_ap.offset + offset_elems,
            ap=[list(d) for d in dims],
        )

    nc = bacc.Bacc("TRN2", target_bir_lowering=False, debug=False,
                   num_devices=NCORES)

    def max3(out_ap, mk_in, step):
        nc.vector.tensor_max(out_ap, mk_in(0), mk_in(step))
        nc.vector.tensor_max(out_ap, out_ap, mk_in(2 * step))

    # ---- DRAM I/O ----
    xcol_d = nc.dram_tensor("xcol", [BPC, 3, 121, 3025], BF16, kind="ExternalInput").ap()
    w1c_d = nc.dram_tensor("w1c", [121, 288], BF16, kind="ExternalInput").ap()
    scb_d = nc.dram_tensor("scb", [128, 24], F32, kind="ExternalInput").ap()
    w2c_d = nc.dram_tensor("w2c", [96, 6400], BF16, kind="ExternalInput").ap()
    w3c_d = nc.dram_tensor("w3c", [3, 18, 128, 128], BF16, kind="ExternalInput").ap()
    w4c_d = nc.dram_tensor("w4c", [3, 27, 128, 128], BF16, kind="ExternalInput").ap()
    w5c_d = nc.dram_tensor("w5c", [2, 27, 128, 128], BF16, kind="ExternalInput").ap()
    fw1_d = nc.dram_tensor("fw1t", [9216, 512], BF16, kind="ExternalInput").ap()
    fb1_d = nc.dram_tensor("fb1r", [1, 512], BF16, kind="ExternalInput").ap()
    fw2_d = nc.dram_tensor("fw2t", [4096, 512], BF16, kind="ExternalInput").ap()
    fb2_d = nc.dram_tensor("fb2r", [1, 512], BF16, kind="ExternalInput").ap()
    fw3_d = nc.dram_tensor("fw3t", [4096, 126], BF16, kind="ExternalInput").ap()
    fb3_d = nc.dram_tensor("fb3r", [1, 126], F32, kind="ExternalInput").ap()
    out_d = nc.dram_tensor("out", [64, 126], F32, kind="ExternalOutput").ap()

    # conv1: DMA chunks per image over the 3025 output positions (first one
    # small so the very first matmul starts early), sub-sliced to <=512
    C1HALF = [(0, 512), (512, 1536), (1536, 2560), (2560, 3025)]

    with tile.TileContext(nc) as tc:
        with tc.tile_pool(name="wp", bufs=1) as wp, \
             tc.tile_pool(name="fcw", bufs=1) as fcw, \
             tc.tile_pool(name="dr", bufs=1, space="DRAM") as dr:

            # ---- persistent small tiles (w1 first so conv1 starts asap) ----
            w1all = wp.tile([121, 288], BF16, name="w1all")
            nc.sync.dma_start(out=w1all, in_=w1c_d)
            scb = wp.tile([128, 24], F32, name="scb")
            nc.gpsimd.dma_start(out=scb, in_=scb_d)

            # sc tile order in blob: sc1(1), sc2(2), sc3(3), sc4(3), sc5(2)
            _SC0 = {"sc1": 0, "sc2": 1, "sc3": 3, "sc4": 6, "sc5": 9}

            def sc_scale(nm, m):
                t = _SC0[nm] + m
                return scb[:, 2 * t:2 * t + 1]

            def sc_bias(nm, m):
                t = _SC0[nm] + m
                return scb[:, 2 * t + 1:2 * t + 2]

            w2all = wp.tile([96, 6400], BF16, name="w2all")
            nc.gpsimd.dma_start(out=w2all, in_=w2c_d)

            fb1r = wp.tile([64, 512], BF16, name="fb1r")
            nc.gpsimd.dma_start(out=fb1r, in_=bass.AP(
                tensor=fb1_d.tensor, offset=0, ap=[[0, 64], [1, 512]]))
            fb2r = wp.tile([64, 512], BF16, name="fb2r")
            nc.gpsimd.dma_start(out=fb2r, in_=bass.AP(
                tensor=fb2_d.tensor, offset=0, ap=[[0, 64], [1, 512]]))
            fb3r = wp.tile([64, 126], F32, name="fb3r")
            nc.gpsimd.dma_start(out=fb3r, in_=bass.AP(
                tensor=fb3_d.tensor, offset=0, ap=[[0, 64], [1, 126]]))

            idn = wp.tile([64, 64], BF16, name="idn")
            make_identity(nc, idn)
            zk = wp.tile([128, 1], F32, name="zk")
            nc.vector.memset(zk, 0.0)

            # tiny warm-up AllGather: absorbs the ~40us cold-start of the
            # collective path while conv1 runs
            warm_in = dr.tile([128, 1], F32, name="warm_in")
            warm_out = dr.tile([128 * NCORES, 1], F32, addr_space="Shared",
                               name="warm_out")
            nc.gpsimd.dma_start(out=warm_in[:, :], in_=zk)
            nc.gpsimd.collective_compute(
                "AllGather", mybir.AluOpType.bypass,
                replica_groups=[list(range(NCORES))],
                ins=[warm_in[:, :]], outs=[warm_out[:, :]])

            def zfill(t, n):
                bc = bass.AP(tensor=zk.tensor, offset=zk.offset,
                             ap=[[zk.ap[0][0], t.ap[0][1]], [0, n]])
                nc.scalar.activation(t, bc, AF.Copy, scale=0.0)

            h_loc = [dr.tile([BPC, 4608], BF16, name=f"h_loc{m}") for m in range(2)]
            h_all = [dr.tile([B, 4608], BF16, addr_space="Shared", name=f"h_all{m}")
                     for m in range(2)]
            y1locT = dr.tile([512, 64], BF16, name="y1locT")
            y1allT = dr.tile([4096, 64], BF16, addr_space="Shared", name="y1allT")
            y2locT = dr.tile([512, 64], BF16, name="y2locT")
            y2allT = dr.tile([4096, 64], BF16, addr_space="Shared", name="y2allT")

            # fw1 group loader: 5 rotating slots. Groups 0-4 prefetch on the
            # gpsimd queue during conv (no slot reuse there — a WAR-stalled
            # DMA would block the AllGather triggers queued behind it);
            # group 5 reuses a slot, so it must load on the scalar queue in
            # the FC section (after the AG triggers).
            def load_fw1(gi, eng):
                wt = fcw.tile([128, 12 * 512], BF16, tag="fw1", bufs=5,
                              name=f"fw1g{gi}")
                eng.dma_start(
                    out=wt,
                    in_=bass.AP(tensor=fw1_d.tensor,
                                offset=gi * 12 * 128 * 512,
                                ap=[[512, 128], [128 * 512, 12], [1, 512]]))
                return wt

            # ================= conv stack =================
            with tc.tile_pool(name="xc", bufs=6) as xc, \
                 tc.tile_pool(name="wq", bufs=2) as wq, \
                 tc.tile_pool(name="act", bufs=1) as act, \
                 tc.tile_pool(name="ps", bufs=6, space="PSUM") as ps:

                c3in = {}
                c2ins = {}

                def conv1_pool1(i):
                    g, islot = divmod(i, 2)
                    c1out = act.tile([96, 3025], BF16, tag="c1out", bufs=1,
                                     name=f"c1out_{i}")
                    for (h0, h1) in C1HALF:
                        hn = h1 - h0
                        xt = xc.tile([121, 3, 1024], BF16, tag="xc")
                        nc.sync.dma_start(
                            out=xt[:, :, :hn],
                            in_=bass.AP(tensor=xcol_d.tensor,
                                        offset=i * 3 * 121 * 3025 + h0,
                                        ap=[[3025, 121], [121 * 3025, 3], [1, hn]]))
                        # c outer: stationary weight reused across sub-slices
                        slices = [(s0, min(512, hn - s0))
                                  for s0 in range(0, hn, 512)]
                        pts = {}
                        for s0, _ in slices:
                            pts[s0] = ps.tile([128, 512], F32, tag="psc1",
                                              bufs=2, name=f"ptc1_{i}_{h0}_{s0}")
                        for c in range(3):
                            for s0, n in slices:
                                nc.tensor.matmul(
                                    pts[s0][:96, :n],
                                    w1all[:, c * 96:(c + 1) * 96],
                                    xt[:, c, s0:s0 + n],
                                    start=(c == 0), stop=(c == 2))
                        for s0, n in slices:
                            nc.scalar.activation(c1out[:, h0 + s0:h0 + s0 + n],
                                                 pts[s0][:96, :n],
                                                 AF.Relu, bias=scb[:96, 1:2],
                                                 scale=1.0)

                    p1 = act.tile([96, 55 * 27], BF16, tag="p1", bufs=2,
                                  name=f"p1_{i}")
                    max3(mkap(p1, 0, [[27, 55], [1, 27]]),
                         lambda o: mkap(c1out, o, [[55, 55], [2, 27]]), 1)
                    if islot == 0:
                        c2in = act.tile([96, 2 * 31 * 32], BF16, tag="c2in", bufs=2,
                                        name=f"c2in_{g}")
                        zfill(c2in, 2 * 31 * 32)
                        c2ins[g] = c2in
                        c3in[g] = [act.tile([128, 2 * 240], BF16, tag=f"padA{kb}_{g}",
                                            name=f"c3in{kb}_{g}")
                                   for kb in range(2)]
                        for t in c3in[g]:
                            zfill(t, 2 * 240)
                    c2in = c2ins[g]
                    c2wv = mkap(c2in, islot * 992 + 2 * 32 + 2, [[1, 27], [32, 27]])
                    nc.vector.tensor_max(c2wv, mkap(p1, 0, [[1, 27], [54, 27]]),
                                         mkap(p1, 27, [[1, 27], [54, 27]]))
                    nc.vector.tensor_max(c2wv, c2wv, mkap(p1, 54, [[1, 27], [54, 27]]))

                def conv2_pool2(g):
                    c2in = c2ins[g]
                    c2pt = {}
                    for m in range(2):
                        for y0 in (0, 9, 18):
                            c2pt[m, y0] = ps.tile([128, 512], F32, tag="ps",
                                                  name=f"c2pt{m}_{y0}")
                    # m then y0 inside sh: the stationary weight tile stays
                    # loaded across the 3 y0 matmuls (conv3/4/5-style reuse)
                    for sh in range(25):
                        r, s = divmod(sh, 5)
                        for m in range(2):
                            for y0 in (0, 9, 18):
                                rhs = mkap(c2in, (y0 + r) * 32 + s,
                                           [[992, 2], [32, 9], [1, 28]])
                                nc.tensor.matmul(
                                    c2pt[m, y0][:, :504],
                                    w2all[:, sh * 256 + m * 128:
                                          sh * 256 + (m + 1) * 128],
                                    rhs, start=(sh == 0), stop=(sh == 24))
                    for m in range(2):
                        co = act.tile([128, 2 * 27 * 28], BF16, tag="c2out", bufs=2)
                        for y0 in (0, 9, 18):
                            pt = c2pt[m, y0]
                            nc.scalar.activation(
                                mkap(co, y0 * 28, [[756, 2], [28, 9], [1, 28]]),
                                mkap(pt, 0, [[252, 2], [28, 9], [1, 28]]),
                                AF.Relu, bias=sc_bias("sc2", m),
                                scale=sc_scale("sc2", m))
                        p2 = act.tile([128, 2 * 27 * 13], BF16, tag="p2", bufs=2)
                        max3(mkap(p2, 0, [[351, 2], [13, 27], [1, 13]]),
                             lambda o: mkap(co, o, [[756, 2], [28, 27], [2, 13]]), 1)
                        c3wv = mkap(c3in[g][m], 17, [[240, 2], [1, 13], [16, 13]])
                        nc.vector.tensor_max(
                            c3wv, mkap(p2, 0, [[351, 2], [1, 13], [26, 13]]),
                            mkap(p2, 13, [[351, 2], [1, 13], [26, 13]]))
                        nc.vector.tensor_max(
                            c3wv, c3wv, mkap(p2, 26, [[351, 2], [1, 13], [26, 13]]))

                for g in range(4):
                    conv1_pool1(2 * g)
                    conv1_pool1(2 * g + 1)
                    if g >= 1:
                        conv2_pool2(g - 1)
                conv2_pool2(3)

                # prefetch fw1 during conv3/4/5 on the scalar queue: issued
                # only after conv2's epilogues, so the transfers don't steal
                # HBM bandwidth from the startup xcol stream
                fw1g = {gi: load_fw1(gi, nc.scalar) for gi in range(5)}

                # ---- layer-wise conv3/4/5 over the 4 image-pairs ----
                def make_padded(tag, nblk, tags=None):
                    outs = {g: [act.tile([128, 2 * 240], BF16,
                                         tag=(tags[m] + f"_{g}" if tags
                                              else f"{tag}{m}_{g}"),
                                         name=f"{tag}{m}_{g}")
                                for m in range(nblk)] for g in range(4)}
                    for g in range(4):
                        for t in outs[g]:
                            zfill(t, 2 * 240)
                    return outs

                def convq(cins, w_d, scnm, nkb, M, write_fn, post_m=None):
                    nt = 9 * nkb
                    for m in range(M // 128):
                        pts = [ps.tile([128, 512], F32, tag="ps",
                                       name=f"ptq{m}_{g}") for g in range(4)]
                        wqt = wq.tile([128, nt * 128], BF16, tag="wq", bufs=2)
                        nc.sync.dma_start(
                            out=wqt,
                            in_=bass.AP(tensor=w_d.tensor,
                                        offset=m * nt * 128 * 128,
                                        ap=[[128, 128], [128 * 128, nt], [1, 128]]))
                        for sh in range(9):
                            r, s = divmod(sh, 3)
                            for kb in range(nkb):
                                wt = wqt[:, (sh * nkb + kb) * 128:
                                         (sh * nkb + kb + 1) * 128]
                                first = (sh == 0 and kb == 0)
                                last = (sh == 8 and kb == nkb - 1)
                                for g in range(4):
                                    rhs = mkap(cins[g][kb], r * 16 + s,
                                               [[240, 2], [16, 13], [1, 13]])
                                    nc.tensor.matmul(
                                        pts[g][:, :338], wt, rhs,
                                        start=first, stop=last)
                        for g in range(4):
                            write_fn(g, m, pts[g], scnm)
                        if post_m is not None:
                            post_m(m)

                def write_pad(nxt):
                    def fn(g, m, pt, scnm):
                        nc.scalar.activation(
                            mkap(nxt[g][m], 17, [[240, 2], [16, 13], [1, 13]]),
                            mkap(pt, 0, [[169, 2], [13, 13], [1, 13]]),
                            AF.Relu, bias=sc_bias(scnm, m),
                            scale=sc_scale(scnm, m))
                    return fn

                c4in = make_padded("c4in", 3)
                convq(c3in, w3c_d, "sc3", 2, 384, write_pad(c4in))
                # c5in reuses c3in's slots (conv3 is done reading them)
                c5in = make_padded("c5in", 3,
                                   tags=["padA0", "padA1", "padB"])
                convq(c4in, w4c_d, "sc4", 3, 384, write_pad(c5in))

                def write_c5(g, m, pt, scnm):
                    c5o = act.tile([128, 2 * 169], BF16, tag="c5out", bufs=4)
                    nc.scalar.activation(
                        mkap(c5o, 0, [[169, 2], [13, 13], [1, 13]]),
                        mkap(pt, 0, [[169, 2], [13, 13], [1, 13]]),
                        AF.Relu, bias=sc_bias(scnm, m),
                        scale=sc_scale(scnm, m))
                    p3a = act.tile([128, 2 * 13 * 6], BF16, tag="p3a", bufs=2)
                    max3(mkap(p3a, 0, [[78, 2], [6, 13], [1, 6]]),
                         lambda o: mkap(c5o, o, [[169, 2], [13, 13], [2, 6]]), 1)
                    hst = act.tile([128, 2 * 36], BF16, tag="hst", bufs=2)
                    max3(mkap(hst, 0, [[36, 2], [1, 6], [6, 6]]),
                         lambda o: mkap(p3a, o, [[78, 2], [1, 6], [12, 6]]), 6)
                    nc.sync.dma_start(
                        out=rawap(h_loc[m], (2 * g) * 4608,
                                  [[36, 128], [4608, 2], [1, 36]]),
                        in_=hst.rearrange("p (i f) -> p i f", i=2))

                def post_c5(m):
                    # AllGather each h half as soon as its stores are issued:
                    # m=0's collective overlaps conv5's m=1 compute
                    nc.gpsimd.collective_compute(
                        "AllGather", mybir.AluOpType.bypass,
                        replica_groups=[list(range(NCORES))],
                        ins=[h_loc[m][:, :]], outs=[h_all[m][:, :]])

                convq(c5in, w5c_d, "sc5", 3, 256, write_c5, post_m=post_c5)

            # ================= FC stack =================
            with tc.tile_pool(name="fch", bufs=8) as fch, \
                 tc.tile_pool(name="fca", bufs=1) as fca, \
                 tc.tile_pool(name="fcw2", bufs=1) as fcw2, \
                 tc.tile_pool(name="ps2", bufs=4, space="PSUM") as ps2, \
                 tc.tile_pool(name="psm", bufs=1, space="PSUM") as psm:

                h_sb = [fca.tile([64, 4608], BF16, name=f"h_sb{m}")
                        for m in range(2)]
                for m in range(2):
                    nc.sync.dma_start(out=h_sb[m], in_=h_all[m][:, :])

                # last fw1 group + fw2/fw3 on the scalar queue (gpsimd is
                # parked on the AG triggers); transfers overlap h-AG + fc1
                fw1g[5] = load_fw1(5, nc.scalar)
                fw2all = fcw2.tile([128, 32 * 512], BF16, name="fw2all")
                nc.scalar.dma_start(
                    out=fw2all,
                    in_=bass.AP(tensor=fw2_d.tensor, offset=0,
                                ap=[[512, 128], [128 * 512, 32], [1, 512]]))
                fw3all = fcw2.tile([128, 32 * 126], BF16, name="fw3all")
                nc.scalar.dma_start(
                    out=fw3all,
                    in_=bass.AP(tensor=fw3_d.tensor, offset=0,
                                ap=[[126, 128], [128 * 126, 32], [1, 126]]))

                # fc1, per h-half: all 36 transposes first (short PE ops that
                # re-warm the HAM after the skew-absorbing h-AG gap), then a
                # clean 36-matmul stream
                pm1 = [psm.tile([64, 512], F32, tag=f"pm{j}", name=f"pm1_{j}")
                       for j in range(2)]
                hts = {}
                for m in range(2):
                    for off in range(36):
                        kt = m * 36 + off
                        ptr = ps2.tile([128, 64], BF16, tag="ptr")
                        nc.tensor.transpose(
                            ptr, h_sb[m][:, off * 128:(off + 1) * 128], idn)
                        ht = fch.tile([128, 64], BF16, tag="hT", bufs=40,
                                      name=f"hT{kt}")
                        nc.vector.tensor_copy(out=ht, in_=ptr)
                        hts[kt] = ht
                    for off in range(36):
                        kt = m * 36 + off
                        gi, go = divmod(kt, 12)
                        wt = fw1g[gi][:, go * 512:(go + 1) * 512]
                        nc.tensor.matmul(pm1[kt % 2], hts.pop(kt), wt,
                                         start=(kt < 2), stop=(kt >= 70))

                def fc_epilogue(pm, fbr, nout, nm):
                    # DVE may read only one PSUM operand per op
                    yc = fca.tile([64, nout], F32, name=f"{nm}_c")
                    nc.vector.tensor_copy(out=yc, in_=pm[0][:, :nout])
                    ysum = fca.tile([64, nout], F32, name=f"{nm}_sum")
                    nc.vector.tensor_add(ysum, yc, pm[1][:, :nout])
                    yb = fca.tile([64, nout], F32, name=f"{nm}_b")
                    nc.vector.tensor_add(yb, ysum, fbr[:, :nout])
                    yr = fca.tile([64, nout], BF16, name=f"{nm}_r")
                    nc.scalar.activation(yr, yb, AF.Relu)
                    return yr

                def store_T(yr, ylocT, nm):
                    for m in range(4):
                        ptr = ps2.tile([128, 64], BF16, tag="ptr")
                        nc.tensor.transpose(ptr, yr[:, m * 128:(m + 1) * 128], idn)
                        yt = fca.tile([128, 64], BF16, name=f"{nm}_t{m}")
                        nc.vector.tensor_copy(out=yt, in_=ptr)
                        nc.sync.dma_start(out=ylocT[m * 128:(m + 1) * 128, :],
                                          in_=yt)

                y1r = fc_epilogue(pm1, fb1r, 512, "y1")
                store_T(y1r, y1locT, "y1")
                nc.gpsimd.collective_compute(
                    "AllGather", mybir.AluOpType.bypass,
                    replica_groups=[list(range(NCORES))],
                    ins=[y1locT[:, :]], outs=[y1allT[:, :]])

                # fc2
                pm2 = [psm.tile([64, 512], F32, tag=f"pm{2 + j}", name=f"pm2_{j}")
                       for j in range(2)]
                y1sb = []
                for gi in range(4):
                    t = fch.tile([128, 8 * 64], BF16, tag="ysb", bufs=4,
                                 name=f"y1sb{gi}")
                    nc.sync.dma_start(
                        out=t,
                        in_=bass.AP(tensor=y1allT.tensor,
                                    offset=y1allT.offset + gi * 8 * 128 * 64,
                                    ap=[[64, 128], [128 * 64, 8], [1, 64]]))
                    y1sb.append(t)
                for kt in range(32):
                    gi, go = divmod(kt, 8)
                    nc.tensor.matmul(pm2[kt % 2],
                                     y1sb[gi][:, go * 64:(go + 1) * 64],
                                     fw2all[:, kt * 512:(kt + 1) * 512],
                                     start=(kt < 2), stop=(kt >= 30))

                y2r = fc_epilogue(pm2, fb2r, 512, "y2")
                store_T(y2r, y2locT, "y2")
                nc.gpsimd.collective_compute(
                    "AllGather", mybir.AluOpType.bypass,
                    replica_groups=[list(range(NCORES))],
                    ins=[y2locT[:, :]], outs=[y2allT[:, :]])

                # fc3
                pm3 = [psm.tile([64, 126], F32, tag=f"pm{j}", name=f"pm3_{j}")
                       for j in range(2)]
                y2sb = []
                for gi in range(4):
                    t = fch.tile([128, 8 * 64], BF16, tag="ysb", bufs=4,
                                 name=f"y2sb{gi}")
                    nc.sync.dma_start(
                        out=t,
                        in_=bass.AP(tensor=y2allT.tensor,
                                    offset=y2allT.offset + gi * 8 * 128 * 64,
                                    ap=[[64, 128], [128 * 64, 8], [1, 64]]))
                    y2sb.append(t)
                for kt in range(32):
                    gi, go = divmod(kt, 8)
                    nc.tensor.matmul(pm3[kt % 2],
                                     y2sb[gi][:, go * 64:(go + 1) * 64],
                                     fw3all[:, kt * 126:(kt + 1) * 126],
                                     start=(kt < 2), stop=(kt >= 30))

                oc = fca.tile([64, 126], F32, name="oc")
                nc.vector.tensor_copy(out=oc, in_=pm3[0])
                osum = fca.tile([64, 126], F32, name="osum")
                nc.vector.tensor_add(osum, oc, pm3[1])
                osb = fca.tile([64, 126], F32, name="osb")
                # out = E1*E2*(y2' @ fw3T) + fb3
                nc.vector.scalar_tensor_tensor(
                    out=osb, in0=osum, scalar=scb[:64, 22:23],
                    in1=fb3r,
                    op0=mybir.AluOpType.mult, op1=mybir.AluOpType.add)
                nc.sync.dma_start(out=out_d, in_=osb)

    nc.compile()
    return nc


def _get_program():
    if "nc" not in _PROG_CACHE:
        _PROG_CACHE["nc"] = _build_program()
    return _PROG_CACHE["nc"]


def _host_prep(inputs):
    eps = 1e-5
    f32 = np.float32

    def inv(g, v):
        return (g / np.sqrt(v + eps)).astype(f32)

    def rms(w):
        return np.sqrt(np.mean(w.astype(np.float64) ** 2)).astype(f32)

    x = inputs["x"]
    w1, b1 = inputs["w1"], inputs["b1"]
    inv1 = inv(inputs["g1"], inputs["v1"])
    w1f = (w1 * inv1[:, None, None, None]).astype(f32)
    b1f = (b1 * inv1 + inputs["be1"] - inputs["m1"] * inv1).astype(f32)

    # conv1 im2col: [B, 3, 121, 3025] bf16
    xp = np.pad(x, ((0, 0), (0, 0), (2, 2), (2, 2)))
    s = xp.strides
    win = np.lib.stride_tricks.as_strided(
        xp, shape=(B, 3, 11, 11, 55, 55),
        strides=(s[0], s[1], s[2], s[3], 4 * s[2], 4 * s[3]))
    xcol = np.ascontiguousarray(win.reshape(B, 3, 121, 3025)).astype(BF)
    # w1c packed [121, 3, 96] -> [121, 288]
    w1c = np.ascontiguousarray(
        w1f.reshape(96, 3, 121).transpose(2, 1, 0)).reshape(121, 288).astype(BF)

    inv2 = inv(inputs["g2"], inputs["v2"])
    E2 = rms(inputs["w2"])
    sgn2 = np.sign(inputs["w2"]).astype(f32)  # [256, 96, 5, 5]
    # w2c packed [96, 25, 256] -> [96, 6400]
    w2c = np.ascontiguousarray(
        sgn2.reshape(256, 96, 25).transpose(1, 2, 0)).reshape(96, 6400).astype(BF)

    def conv_sgn(w, nkb, M):
        # packed [M//128, 9*nkb, 128, 128]: one contiguous block per m-tile
        sgn = np.sign(w).astype(f32)  # [M, K, 3, 3]
        K = sgn.shape[1]
        out = np.zeros((M // 128, 9 * nkb, 128, 128), f32)
        for r in range(3):
            for s_ in range(3):
                blk = sgn[:, :, r, s_].T  # [K, M]
                for kb in range(nkb):
                    kk = min(128, K - kb * 128)
                    for m in range(M // 128):
                        out[m, (r * 3 + s_) * nkb + kb, :kk] = \
                            blk[kb * 128:kb * 128 + kk, m * 128:(m + 1) * 128]
        return out.astype(BF)

    E3, E4, E5 = rms(inputs["w3"]), rms(inputs["w4"]), rms(inputs["w5"])
    w3c = conv_sgn(inputs["w3"], 2, 384)
    w4c = conv_sgn(inputs["w4"], 3, 384)
    w5c = conv_sgn(inputs["w5"], 3, 256)

    Ef1, Ef2 = rms(inputs["fw1"]), rms(inputs["fw2"])
    sgnf1 = np.sign(inputs["fw1"]).astype(BF)
    sgnf2 = np.sign(inputs["fw2"]).astype(BF)

    # sc blob [128, 24]: 11 (scale, bias) column-pairs + E1*E2 in col 22
    scb = np.zeros((128, 24), f32)

    def put_sc(t, scale, bias):
        M = scale.shape[0]
        for m in range((M + 127) // 128):
            mm = min(128, M - m * 128)
            scb[:mm, 2 * (t + m)] = scale[m * 128:m * 128 + mm]
            scb[:mm, 2 * (t + m) + 1] = bias[m * 128:m * 128 + mm]

    put_sc(0, np.ones(96, f32), b1f)
    put_sc(1, np.full(256, E2 * 1.0, f32) * inv2,
           (inputs["b2"] * inv2 + inputs["be2"] - inputs["m2"] * inv2).astype(f32))
    put_sc(3, np.full(384, E3, f32), inputs["b3"].astype(f32))
    put_sc(6, np.full(384, E4, f32), inputs["b4"].astype(f32))
    put_sc(9, np.full(256, E5, f32), inputs["b5"].astype(f32))
    scb[:, 22] = Ef1 * Ef2

    shared = dict(w1c=w1c, scb=scb, w2c=w2c, w3c=w3c, w4c=w4c, w5c=w5c)
    in_maps = []
    for c in range(NCORES):
        m = dict(shared)
        m["xcol"] = np.ascontiguousarray(xcol[c * BPC:(c + 1) * BPC])
        m["fw1t"] = np.ascontiguousarray(sgnf1[c * 512:(c + 1) * 512].T)
        # fb1' = fb1/E1 (E1 folded forward)
        m["fb1r"] = (inputs["fb1"][c * 512:(c + 1) * 512] / Ef1
                     ).astype(BF).reshape(1, 512)
        m["fw2t"] = np.ascontiguousarray(sgnf2[c * 512:(c + 1) * 512].T)
        m["fb2r"] = (inputs["fb2"][c * 512:(c + 1) * 512] / (Ef1 * Ef2)
                     ).astype(BF).reshape(1, 512)
        fw3s = np.zeros((4096, 126), BF)
        fw3s[:, :125] = inputs["fw3"][c * 125:(c + 1) * 125].T.astype(BF)
        m["fw3t"] = fw3s
        fb3s = np.zeros((1, 126), f32)
        fb3s[0, :125] = inputs["fb3"][c * 125:(c + 1) * 125]
        m["fb3r"] = fb3s
        in_maps.append(m)
    return in_maps


def kernel(**inputs):
    global LAST_EXEC_NS, LAST_RESULTS
    from concourse import bass_utils

    nc = _get_program()
    in_maps = _host_prep(inputs)
    trace = os.environ.get("BASS_KERNEL_TRACE", "0") == "1"
    res = bass_utils.run_bass_kernel_spmd(
        nc, in_maps, core_ids=list(range(NCORES)), trace=trace)
    LAST_EXEC_NS = res.exec_time_ns
    LAST_RESULTS = res

    out = np.zeros((B, 1000), np.float32)
    for c in range(NCORES):
        out[:, c * 125:(c + 1) * 125] = res.results[c]["out"][:, :125]
    return out


# revision 39
# speedup vs baseline: 1.0965x; 1.0965x over previous
"""Quantized-AlexNet forward on 8 trn2 NeuronCores.

Strategy (v2, bf16):
  - data-parallel conv stack: 8 images per core
  - the whole matmul path runs in bf16: quantized weights are exactly
    representable (+-1 signs), activation rounding ~0.1%/layer, well
    inside the 2e-2 budget. bf16 gives 4x on conv1 (vs fp32 matmul),
    2x LDWEIGHTS via FWL, and halves all DMA traffic.
  - conv1: host im2col (bf16), 2 DMA chunks/image
  - conv2..5: shifted-window matmuls, DoReFa scale E + BN folded into
    fp32 epilogue scale/bias (ACT engine), outputs written bf16
  - maxpools: 2-pass DVE pool
  - FC: tensor-parallel over outputs; E1/E2 folded algebraically so
    epilogues stay cheap; collectives batched (2 AG for h halves
    overlapped with conv5, 1 AG each for y1/y2); fw1 prefetched on the
    gpsimd queue during the conv stack.
"""

import os
import numpy as np
import ml_dtypes

BF = ml_dtypes.bfloat16
NCORES = 8
B = 64
BPC = B // NCORES  # images per core

_PROG_CACHE = {}
LAST_EXEC_NS = None
LAST_RESULTS = None


def _build_program():
    import concourse.bass as bass
    import concourse.mybir as mybir
    import concourse.tile as tile
    from concourse import bacc
    from concourse.masks import make_identity

    F32 = mybir.dt.float32
    BF16 = mybir.dt.bfloat16
    AF = mybir.ActivationFunctionType

    def mkap(tile_ap, offset_elems, dims):
        part = tile_ap.ap[0]
        return bass.AP(
            tensor=tile_ap.tensor,
            offset=tile_ap.offset + offset_elems,
            ap=[list(part)] + [list(d) for d in dims],
        )

    def rawap(tile_ap, offset_elems, dims):
        return bass.AP(
            tensor=tile_ap.tensor,
            offset=tile_ap.offset + offset_elems,
            ap=[list(d) for d in dims],
        )

    nc = bacc.Bacc("TRN2", target_bir_lowering=False, debug=False,
                   num_devices=NCORES)

    def max3(out_ap, mk_in, step):
        nc.vector.tensor_max(out_ap, mk_in(0), mk_in(step))
        nc.vector.tensor_max(out_ap, out_ap, mk_in(2 * step))

    # ---- DRAM I/O ----
    xcol_d = nc.dram_tensor("xcol", [BPC, 3, 121, 3025], BF16, kind="ExternalInput").ap()
    w1c_d = nc.dram_tensor("w1c", [121, 288], BF16, kind="ExternalInput").ap()
    scb_d = nc.dram_tensor("scb", [128, 24], F32, kind="ExternalInput").ap()
    w2c_d = nc.dram_tensor("w2c", [96, 6400], BF16, kind="ExternalInput").ap()
    w3c_d = nc.dram_tensor("w3c", [3, 18, 128, 128], BF16, kind="ExternalInput").ap()
    w4c_d = nc.dram_tensor("w4c", [3, 27, 128, 128], BF16, kind="ExternalInput").ap()
    w5c_d = nc.dram_tensor("w5c", [2, 27, 128, 128], BF16, kind="ExternalInput").ap()
    fw1_d = nc.dram_tensor("fw1t", [9216, 512], BF16, kind="ExternalInput").ap()
    fb1_d = nc.dram_tensor("fb1r", [1, 512], BF16, kind="ExternalInput").ap()
    fw2_d = nc.dram_tensor("fw2t", [4096, 512], BF16, kind="ExternalInput").ap()
    fb2_d = nc.dram_tensor("fb2r", [1, 512], BF16, kind="ExternalInput").ap()
    fw3_d = nc.dram_tensor("fw3t", [4096, 126], BF16, kind="ExternalInput").ap()
    fb3_d = nc.dram_tensor("fb3r", [1, 126], F32, kind="ExternalInput").ap()
    out_d = nc.dram_tensor("out", [64, 126], F32, kind="ExternalOutput").ap()

    # conv1: DMA chunks per image over the 3025 output positions (first one
    # small so the very first matmul starts early), sub-sliced to <=512
    C1HALF = [(0, 512), (512, 1536), (1536, 2560), (2560, 3025)]

    with tile.TileContext(nc) as tc:
        with tc.tile_pool(name="wp", bufs=1) as wp, \
             tc.tile_pool(name="fcw", bufs=1) as fcw, \
             tc.tile_pool(name="dr", bufs=1, space="DRAM") as dr:

            # ---- persistent small tiles (w1 first so conv1 starts asap) ----
            w1all = wp.tile([121, 288], BF16, name="w1all")
            nc.sync.dma_start(out=w1all, in_=w1c_d)
            scb = wp.tile([128, 24], F32, name="scb")
            nc.gpsimd.dma_start(out=scb, in_=scb_d)

            # sc tile order in blob: sc1(1), sc2(2), sc3(3), sc4(3), sc5(2)
            _SC0 = {"sc1": 0, "sc2": 1, "sc3": 3, "sc4": 6, "sc5": 9}

            def sc_scale(nm, m):
                t = _SC0[nm] + m
                return scb[:, 2 * t:2 * t + 1]

            def sc_bias(nm, m):
                t = _SC0[nm] + m
                return scb[:, 2 * t + 1:2 * t + 2]

            w2all = wp.tile([96, 6400], BF16, name="w2all")
            nc.gpsimd.dma_start(out=w2all, in_=w2c_d)

            fb1r = wp.tile([64, 512], BF16, name="fb1r")
            nc.gpsimd.dma_start(out=fb1r, in_=bass.AP(
                tensor=fb1_d.tensor, offset=0, ap=[[0, 64], [1, 512]]))
            fb2r = wp.tile([64, 512], BF16, name="fb2r")
            nc.gpsimd.dma_start(out=fb2r, in_=bass.AP(
                tensor=fb2_d.tensor, offset=0, ap=[[0, 64], [1, 512]]))
            fb3r = wp.tile([64, 126], F32, name="fb3r")
            nc.gpsimd.dma_start(out=fb3r, in_=bass.AP(
                tensor=fb3_d.tensor, offset=0, ap=[[0, 64], [1, 126]]))

            idn = wp.tile([64, 64], BF16, name="idn")
            make_identity(nc, idn)
            zk = wp.tile([128, 1], F32, name="zk")
            nc.vector.memset(zk, 0.0)

            # tiny warm-up AllGather: absorbs the ~40us cold-start of the
            # collective path while conv1 runs
            warm_in = dr.tile([128, 1], F32, name="warm_in")
            warm_out = dr.tile([128 * NCORES, 1], F32, addr_space="Shared",
                               name="warm_out")
            nc.gpsimd.dma_start(out=warm_in[:, :], in_=zk)
            nc.gpsimd.collective_compute(
                "AllGather", mybir.AluOpType.bypass,
                replica_groups=[list(range(NCORES))],
                ins=[warm_in[:, :]], outs=[warm_out[:, :]])

            def zfill(t, n):
                bc = bass.AP(tensor=zk.tensor, offset=zk.offset,
                             ap=[[zk.ap[0][0], t.ap[0][1]], [0, n]])
                nc.scalar.activation(t, bc, AF.Copy, scale=0.0)

            h_loc = [dr.tile([BPC, 4608], BF16, name=f"h_loc{m}") for m in range(2)]
            h_all = [dr.tile([B, 4608], BF16, addr_space="Shared", name=f"h_all{m}")
                     for m in range(2)]
            y1locT = dr.tile([512, 64], BF16, name="y1locT")
            y1allT = dr.tile([4096, 64], BF16, addr_space="Shared", name="y1allT")
            y2locT = dr.tile([512, 64], BF16, name="y2locT")
            y2allT = dr.tile([4096, 64], BF16, addr_space="Shared", name="y2allT")

            # fw1 group loader: 5 rotating slots. Groups 0-4 prefetch on the
            # gpsimd queue during conv (no slot reuse there — a WAR-stalled
            # DMA would block the AllGather triggers queued behind it);
            # group 5 reuses a slot, so it must load on the scalar queue in
            # the FC section (after the AG triggers).
            def load_fw1(gi, eng):
                wt = fcw.tile([128, 12 * 512], BF16, tag="fw1", bufs=5,
                              name=f"fw1g{gi}")
                eng.dma_start(
                    out=wt,
                    in_=bass.AP(tensor=fw1_d.tensor,
                                offset=gi * 12 * 128 * 512,
                                ap=[[512, 128], [128 * 512, 12], [1, 512]]))
                return wt

            # ================= conv stack =================
            with tc.tile_pool(name="xc", bufs=6) as xc, \
                 tc.tile_pool(name="wq", bufs=2) as wq, \
                 tc.tile_pool(name="act", bufs=1) as act, \
                 tc.tile_pool(name="ps", bufs=6, space="PSUM") as ps:

                c3in = {}
                c2ins = {}

                def conv1_pool1(i):
                    g, islot = divmod(i, 2)
                    c1out = act.tile([96, 3025], BF16, tag="c1out", bufs=1,
                                     name=f"c1out_{i}")
                    for (h0, h1) in C1HALF:
                        hn = h1 - h0
                        xt = xc.tile([121, 3, 1024], BF16, tag="xc")
                        nc.sync.dma_start(
                            out=xt[:, :, :hn],
                            in_=bass.AP(tensor=xcol_d.tensor,
                                        offset=i * 3 * 121 * 3025 + h0,
                                        ap=[[3025, 121], [121 * 3025, 3], [1, hn]]))
                        # c outer: stationary weight reused across sub-slices
                        slices = [(s0, min(512, hn - s0))
                                  for s0 in range(0, hn, 512)]
                        pts = {}
                        for s0, _ in slices:
                            pts[s0] = ps.tile([128, 512], F32, tag="psc1",
                                              bufs=2, name=f"ptc1_{i}_{h0}_{s0}")
                        for c in range(3):
                            for s0, n in slices:
                                nc.tensor.matmul(
                                    pts[s0][:96, :n],
                                    w1all[:, c * 96:(c + 1) * 96],
                                    xt[:, c, s0:s0 + n],
                                    start=(c == 0), stop=(c == 2))
                        for s0, n in slices:
                            nc.scalar.activation(c1out[:, h0 + s0:h0 + s0 + n],
                                                 pts[s0][:96, :n],
                                                 AF.Relu, bias=scb[:96, 1:2],
                                                 scale=1.0)

                    p1 = act.tile([96, 55 * 27], BF16, tag="p1", bufs=2,
                                  name=f"p1_{i}")
                    max3(mkap(p1, 0, [[27, 55], [1, 27]]),
                         lambda o: mkap(c1out, o, [[55, 55], [2, 27]]), 1)
                    if islot == 0:
                        c2in = act.tile([96, 2 * 31 * 32], BF16, tag="c2in", bufs=2,
                                        name=f"c2in_{g}")
                        zfill(c2in, 2 * 31 * 32)
                        c2ins[g] = c2in
                        c3in[g] = [act.tile([128, 2 * 240], BF16, tag=f"padA{kb}_{g}",
                                            name=f"c3in{kb}_{g}")
                                   for kb in range(2)]
                        for t in c3in[g]:
                            zfill(t, 2 * 240)
                    c2in = c2ins[g]
                    c2wv = mkap(c2in, islot * 992 + 2 * 32 + 2, [[1, 27], [32, 27]])
                    nc.vector.tensor_max(c2wv, mkap(p1, 0, [[1, 27], [54, 27]]),
                                         mkap(p1, 27, [[1, 27], [54, 27]]))
                    nc.vector.tensor_max(c2wv, c2wv, mkap(p1, 54, [[1, 27], [54, 27]]))

                def conv2_pool2(g):
                    c2in = c2ins[g]
                    c2pt = {}
                    for m in range(2):
                        for y0 in (0, 9, 18):
                            c2pt[m, y0] = ps.tile([128, 512], F32, tag="ps",
                                                  name=f"c2pt{m}_{y0}")
                    # m then y0 inside sh: the stationary weight tile stays
                    # loaded across the 3 y0 matmuls (conv3/4/5-style reuse)
                    for sh in range(25):
                        r, s = divmod(sh, 5)
                        for m in range(2):
                            for y0 in (0, 9, 18):
                                rhs = mkap(c2in, (y0 + r) * 32 + s,
                                           [[992, 2], [32, 9], [1, 27]])
                                nc.tensor.matmul(
                                    c2pt[m, y0][:, :486],
                                    w2all[:, sh * 256 + m * 128:
                                          sh * 256 + (m + 1) * 128],
                                    rhs, start=(sh == 0), stop=(sh == 24))
                    for m in range(2):
                        co = act.tile([128, 2 * 27 * 27], BF16, tag="c2out", bufs=2)
                        for y0 in (0, 9, 18):
                            pt = c2pt[m, y0]
                            nc.scalar.activation(
                                mkap(co, y0 * 27, [[729, 2], [27, 9], [1, 27]]),
                                mkap(pt, 0, [[243, 2], [27, 9], [1, 27]]),
                                AF.Relu, bias=sc_bias("sc2", m),
                                scale=sc_scale("sc2", m))
                        p2 = act.tile([128, 2 * 27 * 13], BF16, tag="p2", bufs=2)
                        max3(mkap(p2, 0, [[351, 2], [13, 27], [1, 13]]),
                             lambda o: mkap(co, o, [[729, 2], [27, 27], [2, 13]]), 1)
                        c3wv = mkap(c3in[g][m], 17, [[240, 2], [1, 13], [16, 13]])
                        nc.vector.tensor_max(
                            c3wv, mkap(p2, 0, [[351, 2], [1, 13], [26, 13]]),
                            mkap(p2, 13, [[351, 2], [1, 13], [26, 13]]))
                        nc.vector.tensor_max(
                            c3wv, c3wv, mkap(p2, 26, [[351, 2], [1, 13], [26, 13]]))

                for g in range(4):
                    conv1_pool1(2 * g)
                    conv1_pool1(2 * g + 1)
                    if g >= 1:
                        conv2_pool2(g - 1)
                conv2_pool2(3)

                # prefetch fw1 during conv3/4/5 on the scalar queue: issued
                # only after conv2's epilogues, so the transfers don't steal
                # HBM bandwidth from the startup xcol stream
                fw1g = {gi: load_fw1(gi, nc.scalar) for gi in range(5)}

                # ---- layer-wise conv3/4/5 over the 4 image-pairs ----
                def make_padded(tag, nblk, tags=None):
                    outs = {g: [act.tile([128, 2 * 240], BF16,
                                         tag=(tags[m] + f"_{g}" if tags
                                              else f"{tag}{m}_{g}"),
                                         name=f"{tag}{m}_{g}")
                                for m in range(nblk)] for g in range(4)}
                    for g in range(4):
                        for t in outs[g]:
                            zfill(t, 2 * 240)
                    return outs

                def convq(cins, w_d, scnm, nkb, M, write_fn, post_m=None):
                    nt = 9 * nkb
                    for m in range(M // 128):
                        pts = [ps.tile([128, 512], F32, tag="ps",
                                       name=f"ptq{m}_{g}") for g in range(4)]
                        wqt = wq.tile([128, nt * 128], BF16, tag="wq", bufs=2)
                        nc.sync.dma_start(
                            out=wqt,
                            in_=bass.AP(tensor=w_d.tensor,
                                        offset=m * nt * 128 * 128,
                                        ap=[[128, 128], [128 * 128, nt], [1, 128]]))
                        for sh in range(9):
                            r, s = divmod(sh, 3)
                            for kb in range(nkb):
                                wt = wqt[:, (sh * nkb + kb) * 128:
                                         (sh * nkb + kb + 1) * 128]
                                first = (sh == 0 and kb == 0)
                                last = (sh == 8 and kb == nkb - 1)
                                for g in range(4):
                                    rhs = mkap(cins[g][kb], r * 16 + s,
                                               [[240, 2], [16, 13], [1, 13]])
                                    nc.tensor.matmul(
                                        pts[g][:, :338], wt, rhs,
                                        start=first, stop=last)
                        for g in range(4):
                            write_fn(g, m, pts[g], scnm)
                        if post_m is not None:
                            post_m(m)

                def write_pad(nxt):
                    def fn(g, m, pt, scnm):
                        nc.scalar.activation(
                            mkap(nxt[g][m], 17, [[240, 2], [16, 13], [1, 13]]),
                            mkap(pt, 0, [[169, 2], [13, 13], [1, 13]]),
                            AF.Relu, bias=sc_bias(scnm, m),
                            scale=sc_scale(scnm, m))
                    return fn

                c4in = make_padded("c4in", 3)
                convq(c3in, w3c_d, "sc3", 2, 384, write_pad(c4in))
                # c5in reuses c3in's slots (conv3 is done reading them)
                c5in = make_padded("c5in", 3,
                                   tags=["padA0", "padA1", "padB"])
                convq(c4in, w4c_d, "sc4", 3, 384, write_pad(c5in))

                def write_c5(g, m, pt, scnm):
                    c5o = act.tile([128, 2 * 169], BF16, tag="c5out", bufs=4)
                    nc.scalar.activation(
                        mkap(c5o, 0, [[169, 2], [13, 13], [1, 13]]),
                        mkap(pt, 0, [[169, 2], [13, 13], [1, 13]]),
                        AF.Relu, bias=sc_bias(scnm, m),
                        scale=sc_scale(scnm, m))
                    p3a = act.tile([128, 2 * 13 * 6], BF16, tag="p3a", bufs=2)
                    max3(mkap(p3a, 0, [[78, 2], [6, 13], [1, 6]]),
                         lambda o: mkap(c5o, o, [[169, 2], [13, 13], [2, 6]]), 1)
                    hst = act.tile([128, 2 * 36], BF16, tag="hst", bufs=2)
                    max3(mkap(hst, 0, [[36, 2], [1, 6], [6, 6]]),
                         lambda o: mkap(p3a, o, [[78, 2], [1, 6], [12, 6]]), 6)
                    nc.sync.dma_start(
                        out=rawap(h_loc[m], (2 * g) * 4608,
                                  [[36, 128], [4608, 2], [1, 36]]),
                        in_=hst.rearrange("p (i f) -> p i f", i=2))

                def post_c5(m):
                    # AllGather each h half as soon as its stores are issued:
                    # m=0's collective overlaps conv5's m=1 compute
                    nc.gpsimd.collective_compute(
                        "AllGather", mybir.AluOpType.bypass,
                        replica_groups=[list(range(NCORES))],
                        ins=[h_loc[m][:, :]], outs=[h_all[m][:, :]])

                convq(c5in, w5c_d, "sc5", 3, 256, write_c5, post_m=post_c5)

            # ================= FC stack =================
            with tc.tile_pool(name="fch", bufs=8) as fch, \
                 tc.tile_pool(name="fca", bufs=1) as fca, \
                 tc.tile_pool(name="fcw2", bufs=1) as fcw2, \
                 tc.tile_pool(name="ps2", bufs=4, space="PSUM") as ps2, \
                 tc.tile_pool(name="psm", bufs=1, space="PSUM") as psm:

                h_sb = [fca.tile([64, 4608], BF16, name=f"h_sb{m}")
                        for m in range(2)]
                for m in range(2):
                    nc.sync.dma_start(out=h_sb[m], in_=h_all[m][:, :])

                # last fw1 group + fw2/fw3 on the scalar queue (gpsimd is
                # parked on the AG triggers); transfers overlap h-AG + fc1
                fw1g[5] = load_fw1(5, nc.scalar)
                fw2all = fcw2.tile([128, 32 * 512], BF16, name="fw2all")
                nc.scalar.dma_start(
                    out=fw2all,
                    in_=bass.AP(tensor=fw2_d.tensor, offset=0,
                                ap=[[512, 128], [128 * 512, 32], [1, 512]]))
                fw3all = fcw2.tile([128, 32 * 126], BF16, name="fw3all")
                nc.scalar.dma_start(
                    out=fw3all,
                    in_=bass.AP(tensor=fw3_d.tensor, offset=0,
                                ap=[[126, 128], [128 * 126, 32], [1, 126]]))

                # fc1, per h-half: all 36 transposes first (short PE ops that
                # re-warm the HAM after the skew-absorbing h-AG gap), then a
                # clean 36-matmul stream
                pm1 = [psm.tile([64, 512], F32, tag=f"pm{j}", name=f"pm1_{j}")
                       for j in range(2)]
                hts = {}
                for m in range(2):
                    for off in range(36):
                        kt = m * 36 + off
                        ptr = ps2.tile([128, 64], BF16, tag="ptr")
                        nc.tensor.transpose(
                            ptr, h_sb[m][:, off * 128:(off + 1) * 128], idn)
                        ht = fch.tile([128, 64], BF16, tag="hT", bufs=40,
                                      name=f"hT{kt}")
                        nc.vector.tensor_copy(out=ht, in_=ptr)
                        hts[kt] = ht
                    for off in range(36):
                        kt = m * 36 + off
                        gi, go = divmod(kt, 12)
                        wt = fw1g[gi][:, go * 512:(go + 1) * 512]
                        nc.tensor.matmul(pm1[kt % 2], hts.pop(kt), wt,
                                         start=(kt < 2), stop=(kt >= 70))

                def fc_epilogue(pm, fbr, nout, nm):
                    # DVE may read only one PSUM operand per op
                    yc = fca.tile([64, nout], F32, name=f"{nm}_c")
                    nc.vector.tensor_copy(out=yc, in_=pm[0][:, :nout])
                    ysum = fca.tile([64, nout], F32, name=f"{nm}_sum")
                    nc.vector.tensor_add(ysum, yc, pm[1][:, :nout])
                    yb = fca.tile([64, nout], F32, name=f"{nm}_b")
                    nc.vector.tensor_add(yb, ysum, fbr[:, :nout])
                    yr = fca.tile([64, nout], BF16, name=f"{nm}_r")
                    nc.scalar.activation(yr, yb, AF.Relu)
                    return yr

                def store_T(yr, ylocT, nm):
                    for m in range(4):
                        ptr = ps2.tile([128, 64], BF16, tag="ptr")
                        nc.tensor.transpose(ptr, yr[:, m * 128:(m + 1) * 128], idn)
                        yt = fca.tile([128, 64], BF16, name=f"{nm}_t{m}")
                        nc.vector.tensor_copy(out=yt, in_=ptr)
                        nc.sync.dma_start(out=ylocT[m * 128:(m + 1) * 128, :],
                                          in_=yt)

                y1r = fc_epilogue(pm1, fb1r, 512, "y1")
                store_T(y1r, y1locT, "y1")
                nc.gpsimd.collective_compute(
                    "AllGather", mybir.AluOpType.bypass,
                    replica_groups=[list(range(NCORES))],
                    ins=[y1locT[:, :]], outs=[y1allT[:, :]])

                # fc2
                pm2 = [psm.tile([64, 512], F32, tag=f"pm{2 + j}", name=f"pm2_{j}")
                       for j in range(2)]
                y1sb = []
                for gi in range(4):
                    t = fch.tile([128, 8 * 64], BF16, tag="ysb", bufs=4,
                                 name=f"y1sb{gi}")
                    nc.sync.dma_start(
                        out=t,
                        in_=bass.AP(tensor=y1allT.tensor,
                                    offset=y1allT.offset + gi * 8 * 128 * 64,
                                    ap=[[64, 128], [128 * 64, 8], [1, 64]]))
                    y1sb.append(t)
                for kt in range(32):
                    gi, go = divmod(kt, 8)
                    nc.tensor.matmul(pm2[kt % 2],
                                     y1sb[gi][:, go * 64:(go + 1) * 64],
                                     fw2all[:, kt * 512:(kt + 1) * 512],
                                     start=(kt < 2), stop=(kt >= 30))

                y2r = fc_epilogue(pm2, fb2r, 512, "y2")
                store_T(y2r, y2locT, "y2")
                nc.gpsimd.collective_compute(
                    "AllGather", mybir.AluOpType.bypass,
                    replica_groups=[list(range(NCORES))],
                    ins=[y2locT[:, :]], outs=[y2allT[:, :]])

                # fc3
                pm3 = [psm.tile([64, 126], F32, tag=f"pm{j}", name=f"pm3_{j}")
                       for j in range(2)]
                y2sb = []
                for gi in range(4):
                    t = fch.tile([128, 8 * 64], BF16, tag="ysb", bufs=4,
                                 name=f"y2sb{gi}")
                    nc.sync.dma_start(
                        out=t,
                        in_=bass.AP(tensor=y2allT.tensor,
                                    offset=y2allT.offset + gi * 8 * 128 * 64,
                                    ap=[[64, 128], [128 * 64, 8], [1, 64]]))
                    y2sb.append(t)
                for kt in range(32):
                    gi, go = divmod(kt, 8)
                    nc.tensor.matmul(pm3[kt % 2],
                                     y2sb[gi][:, go * 64:(go + 1) * 64],
                                     fw3all[:, kt * 126:(kt + 1) * 126],
                                     start=(kt < 2), stop=(kt >= 30))

                oc = fca.tile([64, 126], F32, name="oc")
                nc.vector.tensor_copy(out=oc, in_=pm3[0])
                osum = fca.tile([64, 126], F32, name="osum")
                nc.vector.tensor_add(osum, oc, pm3[1])
                osb = fca.tile([64, 126], F32, name="osb")
                # out = E1*E2*(y2' @ fw3T) + fb3
                nc.vector.scalar_tensor_tensor(
                    out=osb, in0=osum, scalar=scb[:64, 22:23],
                    in1=fb3r,
                    op0=mybir.AluOpType.mult, op1=mybir.AluOpType.add)
                nc.sync.dma_start(out=out_d, in_=osb)

    nc.compile()
    return nc


def _get_program():
    if "nc" not in _PROG_CACHE:
        _PROG_CACHE["nc"] = _build_program()
    return _PROG_CACHE["nc"]


def _host_prep(inputs):
    eps = 1e-5
    f32 = np.float32

    def inv(g, v):
        return (g / np.sqrt(v + eps)).astype(f32)

    def rms(w):
        return np.sqrt(np.mean(w.astype(np.float64) ** 2)).astype(f32)

    x = inputs["x"]
    w1, b1 = inputs["w1"], inputs["b1"]
    inv1 = inv(inputs["g1"], inputs["v1"])
    w1f = (w1 * inv1[:, None, None, None]).astype(f32)
    b1f = (b1 * inv1 + inputs["be1"] - inputs["m1"] * inv1).astype(f32)

    # conv1 im2col: [B, 3, 121, 3025] bf16
    xp = np.pad(x, ((0, 0), (0, 0), (2, 2), (2, 2)))
    s = xp.strides
    win = np.lib.stride_tricks.as_strided(
        xp, shape=(B, 3, 11, 11, 55, 55),
        strides=(s[0], s[1], s[2], s[3], 4 * s[2], 4 * s[3]))
    xcol = np.ascontiguousarray(win.reshape(B, 3, 121, 3025)).astype(BF)
    # w1c packed [121, 3, 96] -> [121, 288]
    w1c = np.ascontiguousarray(
        w1f.reshape(96, 3, 121).transpose(2, 1, 0)).reshape(121, 288).astype(BF)

    inv2 = inv(inputs["g2"], inputs["v2"])
    E2 = rms(inputs["w2"])
    sgn2 = np.sign(inputs["w2"]).astype(f32)  # [256, 96, 5, 5]
    # w2c packed [96, 25, 256] -> [96, 6400]
    w2c = np.ascontiguousarray(
        sgn2.reshape(256, 96, 25).transpose(1, 2, 0)).reshape(96, 6400).astype(BF)

    def conv_sgn(w, nkb, M):
        # packed [M//128, 9*nkb, 128, 128]: one contiguous block per m-tile
        sgn = np.sign(w).astype(f32)  # [M, K, 3, 3]
        K = sgn.shape[1]
        out = np.zeros((M // 128, 9 * nkb, 128, 128), f32)
        for r in range(3):
            for s_ in range(3):
                blk = sgn[:, :, r, s_].T  # [K, M]
                for kb in range(nkb):
                    kk = min(128, K - kb * 128)
                    for m in range(M // 128):
                        out[m, (r * 3 + s_) * nkb + kb, :kk] = \
                            blk[kb * 128:kb * 128 + kk, m * 128:(m + 1) * 128]
        return out.astype(BF)

    E3, E4, E5 = rms(inputs["w3"]), rms(inputs["w4"]), rms(inputs["w5"])
    w3c = conv_sgn(inputs["w3"], 2, 384)
    w4c = conv_sgn(inputs["w4"], 3, 384)
    w5c = conv_sgn(inputs["w5"], 3, 256)

    Ef1, Ef2 = rms(inputs["fw1"]), rms(inputs["fw2"])
    sgnf1 = np.sign(inputs["fw1"]).astype(BF)
    sgnf2 = np.sign(inputs["fw2"]).astype(BF)

    # sc blob [128, 24]: 11 (scale, bias) column-pairs + E1*E2 in col 22
    scb = np.zeros((128, 24), f32)

    def put_sc(t, scale, bias):
        M = scale.shape[0]
        for m in range((M + 127) // 128):
            mm = min(128, M - m * 128)
            scb[:mm, 2 * (t + m)] = scale[m * 128:m * 128 + mm]
            scb[:mm, 2 * (t + m) + 1] = bias[m * 128:m * 128 + mm]

    put_sc(0, np.ones(96, f32), b1f)
    put_sc(1, np.full(256, E2 * 1.0, f32) * inv2,
           (inputs["b2"] * inv2 + inputs["be2"] - inputs["m2"] * inv2).astype(f32))
    put_sc(3, np.full(384, E3, f32), inputs["b3"].astype(f32))
    put_sc(6, np.full(384, E4, f32), inputs["b4"].astype(f32))
    put_sc(9, np.full(256, E5, f32), inputs["b5"].astype(f32))
    scb[:, 22] = Ef1 * Ef2

    shared = dict(w1c=w1c, scb=scb, w2c=w2c, w3c=w3c, w4c=w4c, w5c=w5c)
    in_maps = []
    for c in range(NCORES):
        m = dict(shared)
        m["xcol"] = np.ascontiguousarray(xcol[c * BPC:(c + 1) * BPC])
        m["fw1t"] = np.ascontiguousarray(sgnf1[c * 512:(c + 1) * 512].T)
        # fb1' = fb1/E1 (E1 folded forward)
        m["fb1r"] = (inputs["fb1"][c * 512:(c + 1) * 512] / Ef1
                     ).astype(BF).reshape(1, 512)
        m["fw2t"] = np.ascontiguousarray(sgnf2[c * 512:(c + 1) * 512].T)
        m["fb2r"] = (inputs["fb2"][c * 512:(c + 1) * 512] / (Ef1 * Ef2)
                     ).astype(BF).reshape(1, 512)
        fw3s = np.zeros((4096, 126), BF)
        fw3s[:, :125] = inputs["fw3"][c * 125:(c + 1) * 125].T.astype(BF)
        m["fw3t"] = fw3s
        fb3s = np.zeros((1, 126), f32)
        fb3s[0, :125] = inputs["fb3"][c * 125:(c + 1) * 125]
        m["fb3r"] = fb3s
        in_maps.append(m)
    return in_maps


def kernel(**inputs):
    global LAST_EXEC_NS, LAST_RESULTS
    from concourse import bass_utils

    nc = _get_program()
    in_maps = _host_prep(inputs)
    trace = os.environ.get("BASS_KERNEL_TRACE", "0") == "1"
    res = bass_utils.run_bass_kernel_spmd(
        nc, in_maps, core_ids=list(range(NCORES)), trace=trace)
    LAST_EXEC_NS = res.exec_time_ns
    LAST_RESULTS = res

    out = np.zeros((B, 1000), np.float32)
    for c in range(NCORES):
        out[:, c * 125:(c + 1) * 125] = res.results[c]["out"][:, :125]
    return out
